# revision 1
# baseline (speedup 1.0000x reference)
# BitStackLinear Trainium2 kernel (8-core column-parallel).
#
# reference computation:
#   sign  = unpack_bits(qweight) in {-1,+1}            [4, 4096, 4096]  (b, o, i)
#   w     = sum_b sign_b * (u_b @ vt_b)                [4096, 4096]     (o, i)
#   out   = x @ w.T                                    [4, 2048, 4096]
#
# Sharding: column-parallel over out_features (512 per core). x replicated.
#
# Per-core device program (single pass, w.T formation pipelined into the
# first token group of the main matmul):
#   For each PAIR of i-tiles (2 x 128 rows of in_f, pairing halves the DVE
#   per-op fixed costs):
#     - L_b.T[i, o] = vt_b.T @ u_b.T via PE (K=16, psum f32) -> fp16 (ScalarE,
#       one wide [128,1024] copy per 2-matmul round)
#     - sign masks m_b in {0, 0x8000} via DVE tensor_scalar(AND, SHL)
#       (host packs INVERTED bits, so m=0x8000 exactly when sign is -1)
#     - prod_b = L_b XOR m_b (uint16 view: flips fp16 sign bit -> exact +-L),
#       one wide [128,4096] DVE op per pair
#     - wT_tile = p0 + p1 + p2 + p3 via gpsimd SWDGE DMA accumulate (frees DVE)
#     - token-group-0 matmuls for each i-tile issue as soon as wT is ready
#   Then remaining token groups: out[t, o] = sum_i xT[i,t].T @ wT[i,o]
#   PSUM budget in the window: 6 banks group-0 accums + 2 banks L psum.
#
# Host prep: transpose x to [in_f, tokens]; repack qweight bits into uint16
# words so that on-device bit l of word j covers output column o = 32*l + j
# (bit-plane-major -> unpacked masks land contiguous in o, no permutation).

import sys

import numpy as np

for p in ("/opt/trn_rl_repo", "/opt/pypackages"):
    if p not in sys.path:
        sys.path.insert(0, p)

import concourse.bacc as bacc
import concourse.mybir as mybir
import concourse.tile as tile
from concourse.bass_utils import run_bass_kernel_spmd

W_BIT, OUT_F, IN_F, K = 4, 4096, 4096, 16
B, S = 4, 2048
T = B * S                      # 8192 tokens
NCORES = 8
OS = OUT_F // NCORES           # 512 out features per core
N_ITILES = IN_F // 128         # 32

# token groups: (start_token, n_ttiles). group 0 runs under formation with 6
# psum banks; the rest use 8; remainder group of 2 closes the books.
GROUPS = [(0, 3)] + [(384 + 1024 * g, 8) for g in range(7)] + [(7552, 5)]

FP16 = mybir.dt.float16
F32 = mybir.dt.float32
U16 = mybir.dt.uint16
Alu = mybir.AluOpType

USE_DMA_ACC = False

_cached = {}


def build_nc():
    nc = bacc.Bacc("TRN2", target_bir_lowering=False, debug=False,
                   num_devices=NCORES)
    xt_p = nc.dram_tensor("xt", [IN_F, T], FP16, kind="ExternalInput").ap()
    qp_p = nc.dram_tensor("qp", [IN_F, 128], U16, kind="ExternalInput").ap()
    ut_p = nc.dram_tensor("ut", [W_BIT, K, OS], FP16, kind="ExternalInput").ap()
    vt_p = nc.dram_tensor("vt4", [W_BIT, K, IN_F], FP16, kind="ExternalInput").ap()
    out_p = nc.dram_tensor("out", [T, OS], FP16, kind="ExternalOutput").ap()

    with tile.TileContext(nc) as tc:
        with (
            tc.tile_pool(name="const", bufs=1) as cpool,
            tc.tile_pool(name="wt", bufs=1) as wtpool,
            tc.tile_pool(name="fq", bufs=16) as fq,
            tc.tile_pool(name="fl", bufs=4) as fl,
            tc.tile_pool(name="fbits", bufs=4) as fb,
            tc.tile_pool(name="fa", bufs=4) as fa,
            tc.tile_pool(name="mx", bufs=8) as mx,
            tc.tile_pool(name="mo", bufs=8) as mo,
        ):
            # resident operands
            vt_b = []
            ut_b = []
            for b in range(W_BIT):
                v = cpool.tile([K, IN_F], FP16, tag=f"vt{b}")
                nc.sync.dma_start(v[:], vt_p[b, :, :])
                vt_b.append(v)
                uu = cpool.tile([K, OS], FP16, tag=f"ut{b}")
                nc.sync.dma_start(uu[:], ut_p[b, :, :])
                ut_b.append(uu)

            # w.T tiles, one per i-tile (separate tiles so main-loop reads of
            # tile it' never falsely depend on formation writes of tile it)
            wts = [
                wtpool.tile([128, OS], FP16, tag=f"wt{it}", name=f"wt_{it}")
                for it in range(N_ITILES)
            ]

            def mm_group(gi, it):
                t0, ntt = GROUPS[gi]
                xs = mx.tile([128, ntt * 128], FP16, tag="x")
                nc.sync.dma_start(
                    xs[:], xt_p[it * 128:(it + 1) * 128, t0:t0 + ntt * 128]
                )
                for tt in range(ntt):
                    nc.tensor.matmul(
                        acc_tiles[tt][:],
                        xs[:, tt * 128:(tt + 1) * 128],
                        wts[it][:],
                        start=(it == 0),
                        stop=(it == N_ITILES - 1),
                    )

            def flush_group(gi):
                t0, ntt = GROUPS[gi]
                for tt in range(ntt):
                    ot = mo.tile([128, OS], FP16, tag="o")
                    nc.scalar.copy(ot[:], acc_tiles[tt][:])
                    r0 = t0 + tt * 128
                    nc.sync.dma_start(out_p[r0:r0 + 128, :], ot[:])

            # ---- formation (i-tile pairs) pipelined with token group 0 ----
            with (
                tc.tile_pool(name="mps0", bufs=3, space="PSUM") as mps0,
                tc.tile_pool(name="psL", bufs=2, space="PSUM") as psL,
            ):
                acc_tiles = [
                    mps0.tile([128, OS], F32, tag="ps", name=f"acc_0_{tt}")
                    for tt in range(GROUPS[0][1])
                ]
                for ip in range(N_ITILES // 2):
                    it0 = 2 * ip
                    # packed sign words for both i-tiles of the pair
                    q = fq.tile([128, 256], U16, tag="q", name=f"q_{ip}")
                    for h in range(2):
                        r0 = (it0 + h) * 128
                        nc.sync.dma_start(
                            q[:, h * 128:(h + 1) * 128], qp_p[r0:r0 + 128, :]
                        )

                    # low-rank psums -> fp16; 4 rounds of 2 matmuls + 1 copy
                    ls = fl.tile([128, 2 * W_BIT * OS], FP16, tag="Ls")
                    for r in range(4):  # (h, b) pairs: (0,0),(0,1)...(1,3)
                        pl = psL.tile([128, 2 * OS], F32, tag="pl",
                                      name=f"pl_{ip}_{r}")
                        for c in range(2):
                            h, b = divmod(2 * r + c, W_BIT)
                            isl = slice((it0 + h) * 128, (it0 + h) * 128 + 128)
                            nc.tensor.matmul(
                                pl[:, c * OS:(c + 1) * OS],
                                vt_b[b][:, isl], ut_b[b][:],
                                start=True, stop=True,
                            )
                        nc.scalar.copy(
                            ls[:, r * 2 * OS:(r + 1) * 2 * OS], pl[:]
                        )

                    # sign masks in {0, 0x8000}: one tensor_scalar per bit l,
                    # covering both halves and all 4 planes (FD=256)
                    masks = fb.tile([128, 2 * W_BIT * OS], U16, tag="masks")
                    q3 = q[:].rearrange("p (h b j) -> p h b j", h=2, b=W_BIT)
                    m5 = masks[:].rearrange(
                        "p (h b l j) -> p h b l j", h=2, b=W_BIT, l=16
                    )
                    for l in range(16):
                        nc.vector.tensor_scalar(
                            m5[:, :, :, l, :], q3, 1 << l, 15 - l,
                            op0=Alu.bitwise_and, op1=Alu.logical_shift_left,
                        )

                    # prod = L ^ m for both halves, all planes: one wide op
                    prods = fa.tile([128, 2 * W_BIT * OS], FP16, tag="prods")
                    nc.vector.tensor_tensor(
                        prods[:].bitcast(U16), ls[:].bitcast(U16), masks[:],
                        op=Alu.bitwise_xor,
                    )

                    # wT = p0 + p1 + p2 + p3 per half
                    for h in range(2):
                        it = it0 + h
                        base = h * W_BIT * OS
                        if USE_DMA_ACC:
                            nc.gpsimd.dma_start(
                                wts[it][:], prods[:, base:base + OS]
                            )
                            for b in range(1, W_BIT):
                                o0 = base + b * OS
                                nc.gpsimd.dma_start(
                                    wts[it][:], prods[:, o0:o0 + OS],
                                    accum_op=Alu.add,
                                )
                        else:
                            p01 = fa.tile([128, 2 * OS], FP16, tag="p01")
                            nc.vector.tensor_add(
                                p01[:], prods[:, base:base + 2 * OS],
                                prods[:, base + 2 * OS:base + 4 * OS],
                            )
                            nc.vector.tensor_add(
                                wts[it][:], p01[:, 0:OS], p01[:, OS:2 * OS]
                            )
                        mm_group(0, it)
                flush_group(0)

            # ---- remaining token groups (full 8 psum banks) ----
            with tc.tile_pool(name="mps", bufs=8, space="PSUM") as mps:
                for gi in range(1, len(GROUPS)):
                    acc_tiles = [
                        mps.tile([128, OS], F32, tag="ps", name=f"acc_{gi}_{tt}")
                        for tt in range(GROUPS[gi][1])
                    ]
                    for it in range(N_ITILES):
                        mm_group(gi, it)
                    flush_group(gi)
    nc.compile()
    return nc


def prep_inputs(x, qweight, u, vt):
    """Host-side shard prep. Returns per-core input maps."""
    x = np.asarray(x, dtype=np.float16)
    qweight = np.asarray(qweight)
    u = np.asarray(u, dtype=np.float16)
    vt = np.ascontiguousarray(np.asarray(vt, dtype=np.float16))

    xt = np.ascontiguousarray(x.reshape(T, IN_F).T)  # [IN_F, T]

    # unpack bits: (b, o, i); INVERT so mask=0x8000 <=> sign -1 (bit 0)
    bytes_ = qweight.astype(np.uint8)
    bits = np.unpackbits(bytes_.reshape(W_BIT, -1, 1), axis=2, bitorder="little")
    bits = bits.reshape(W_BIT, OUT_F, IN_F)
    inv = (1 - bits).astype(np.uint16)
    # word[c][i, b*32 + j] bit l = inv[b, 512c + 32l + j, i]
    bl = inv.reshape(W_BIT, NCORES, 16, 32, IN_F)  # [b, c, l, j, i]
    words = np.zeros((W_BIT, NCORES, 32, IN_F), np.uint16)
    for l in range(16):
        words |= bl[:, :, l, :, :] << np.uint16(l)
    qp_all = words.transpose(1, 3, 0, 2)  # [c, i, b, j]

    in_maps = []
    for c in range(NCORES):
        uc = u[:, c * OS:(c + 1) * OS, :]                 # [4, 512, 16]
        ut = np.ascontiguousarray(uc.transpose(0, 2, 1))  # [4, 16, 512]
        qp_c = np.ascontiguousarray(qp_all[c]).reshape(IN_F, 128)
        in_maps.append({"xt": xt, "qp": qp_c, "ut": ut, "vt4": vt})
    return in_maps


def kernel(x, qweight, u, vt, _trace=False):
    if "nc" not in _cached:
        _cached["nc"] = build_nc()
    nc = _cached["nc"]
    in_maps = prep_inputs(x, qweight, u, vt)
    res = run_bass_kernel_spmd(nc, in_maps, list(range(NCORES)), trace=_trace)
    _cached["last_result"] = res
    out = np.concatenate([res.results[c]["out"] for c in range(NCORES)], axis=1)
    return out.reshape(B, S, OUT_F).astype(np.float16)

